# revision 30
# baseline (speedup 1.0000x reference)
"""Multi-head attention on 8 trn2 NeuronCores, head-parallel (2 heads/core).

Math per head h (reference semantics):
  Q = query @ Wq[h] + bq[h];  K = key @ Wk[h] + bk[h];  V = query @ Wv[h] + bv[h]
  P = exp(Q K^T / sqrt(D));  alpha = P / rowsum(P)
  ctx = alpha @ V;  y_h = (ctx @ Wp[h] + bp[h]) @ Wo[h]
  out = sum_h y_h + bo

Device-side formulation (all layouts transposed, f32 storage, f32r matmuls):
  Per core: project QT/KT/VT = W^T @ xT per head, attention with unnormalized
  softmax (rowsum via ones-matmul, normalization folded after PV), output
  y^T[e, tok] partial = sum_{h in core} W_h^T @ ctxn_h + bias ** ones, where
  W_h = Wp[h] @ Wo[h] (host-premultiplied) and bias collects bv/bp/bo terms.
  ReduceScatter across the 8 cores; host concatenates the shards.
"""

import sys

if "/opt/trn_rl_repo" not in sys.path:
    sys.path.insert(0, "/opt/trn_rl_repo")

import ml_dtypes
import numpy as np

import concourse.mybir as mybir
import concourse.tile as tile
from concourse import bacc
from concourse.bass_utils import run_bass_kernel_spmd

B, S = 4, 2048
IN, D, H = 1024, 128, 16
NCORES = 8
HPC = H // NCORES  # heads per core
NCH = IN // 128  # input chunks
TB = 512  # projection token block
NTB = S // TB
QB = 512  # attention query block
NQB = S // QB
KT = 128  # attention key tile
NKT = S // KT
ESH = D // NCORES  # output shard rows per core

f32 = mybir.dt.float32
f32r = mybir.dt.float32r
bf16 = mybir.dt.bfloat16
AF = mybir.ActivationFunctionType

_cache = {}


def build():
    nc = bacc.Bacc(None, target_bir_lowering=False, num_devices=NCORES)

    qT = nc.dram_tensor("qT", [B, IN, S], bf16, kind="ExternalInput")
    kT = nc.dram_tensor("kT", [B, IN, S], bf16, kind="ExternalInput")
    wq = nc.dram_tensor("wq", [HPC, IN, D], bf16, kind="ExternalInput")
    wk = nc.dram_tensor("wk", [HPC, IN, D], bf16, kind="ExternalInput")
    wv = nc.dram_tensor("wv", [HPC, IN, D], bf16, kind="ExternalInput")
    wh = nc.dram_tensor("wh", [HPC, D, D], f32r, kind="ExternalInput")
    bqT = nc.dram_tensor("bqT", [D, HPC], f32, kind="ExternalInput")
    bkT = nc.dram_tensor("bkT", [D, HPC], f32, kind="ExternalInput")
    biasv = nc.dram_tensor("biasv", [1, D], f32r, kind="ExternalInput")
    oner = nc.dram_tensor("oner", [1, QB], f32r, kind="ExternalInput")
    onemb = nc.dram_tensor("onemb", [D, D], bf16, kind="ExternalInput")

    out_y = nc.dram_tensor("out_y", [ESH, B * S], f32, kind="ExternalOutput")
    y_bounce = [
        [nc.dram_tensor(f"y_bounce{b}_{q}", [D, S // 2], f32) for q in range(2)]
        for b in range(B)
    ]
    y_shard = [
        [nc.dram_tensor(f"y_shard{b}_{q}", [ESH, S // 2], f32) for q in range(2)]
        for b in range(B)
    ]

    scale = 1.0 / float(np.sqrt(D))

    with tile.TileContext(nc) as tc:
        with (
            tc.tile_pool(name="const", bufs=1) as cpool,
            tc.tile_pool(name="xch", bufs=24) as xch,
            tc.tile_pool(name="qkv", bufs=2) as qkv,
            tc.tile_pool(name="work", bufs=2) as work,
            tc.tile_pool(name="pexpp", bufs=4) as pexpp,
            tc.tile_pool(name="ps", bufs=2, space="PSUM") as ps,
        ):
            # ---- resident constants ----
            wq_sb = cpool.tile([128, HPC, NCH, D], bf16, tag="wq_sb")
            wk_sb = cpool.tile([128, HPC, NCH, D], bf16, tag="wk_sb")
            wv_sb = cpool.tile([128, HPC, NCH, D], bf16, tag="wv_sb")
            for sb_t, dram_t in ((wq_sb, wq), (wk_sb, wk), (wv_sb, wv)):
                nc.sync.dma_start(
                    sb_t[:], dram_t[:].rearrange("h (c p) d -> p h c d", p=128)
                )
            wh_sb = cpool.tile([128, HPC, D], f32r, tag="wh_sb")
            nc.sync.dma_start(wh_sb[:], wh[:].rearrange("h d e -> d h e"))
            bq_sb = cpool.tile([128, HPC], f32, tag="bq_sb")
            bk_sb = cpool.tile([128, HPC], f32, tag="bk_sb")
            nc.sync.dma_start(bq_sb[:], bqT[:])
            nc.sync.dma_start(bk_sb[:], bkT[:])
            biasv_sb = cpool.tile([1, D], f32r, tag="biasv_sb")
            oner_sb = cpool.tile([1, QB], f32r, tag="oner_sb")
            onemb_sb = cpool.tile([D, D], bf16, tag="onemb_sb")
            nc.sync.dma_start(onemb_sb[:], onemb[:])
            nc.sync.dma_start(biasv_sb[:], biasv[:])
            nc.sync.dma_start(oner_sb[:], oner[:])

            QTd, KTd, Vnd = {}, {}, {}

            def proj_batch(b):
                # ---- projections: Q & V from qT, K from kT ----
                QT = QTd[b] = [qkv.tile([128, S], bf16, tag=f"QT{h}", name=f"QT{h}") for h in range(HPC)]
                KTs = KTd[b] = [qkv.tile([128, S], bf16, tag=f"KT{h}", name=f"KT{h}") for h in range(HPC)]
                Vn = Vnd[b] = [qkv.tile([128, S], bf16, tag=f"VN{h}", name=f"VN{h}") for h in range(HPC)]

                for tb in range(NTB):
                    sl = slice(tb * TB, (tb + 1) * TB)
                    chs = xch.tile([128, NCH, TB], bf16, tag="xch", bufs=3)
                    nc.sync.dma_start(
                        chs[:], qT[b, :, sl].rearrange("(c p) n -> p c n", p=128)
                    )
                    pq = ps.tile([128, 2 * TB], f32, tag="pS", name="pq", bufs=2)
                    for h in range(HPC):
                        for c in range(NCH):
                            nc.tensor.matmul(
                                pq[:, h * TB : (h + 1) * TB],
                                wq_sb[:, h, c, :], chs[:, c, :],
                                start=(c == 0), stop=(c == NCH - 1),
                            )
                    for h in range(HPC):
                        with nc.allow_low_precision(reason="f32r PE operand"):
                            nc.vector.tensor_scalar_add(
                                QT[h][:, sl], pq[:, h * TB : (h + 1) * TB],
                                bq_sb[:, h : h + 1],
                            )
                    # V in natural [tok, d] layout: chunk subtiles as stationary
                    for t in range(TB // 128):
                        pvt = ps.tile([128, 2 * D], f32, tag="pC", name="pvt", bufs=4)
                        for c in range(NCH):
                            nc.tensor.matmul(
                                pvt[:],
                                chs[:, c, t * 128 : (t + 1) * 128],
                                wv_sb[:, :, c, :],
                                start=(c == 0), stop=(c == NCH - 1),
                            )
                        col = tb * TB + t * 128
                        for h in range(HPC):
                            with nc.allow_low_precision(reason="bf16 PV operand"):
                                nc.vector.tensor_copy(
                                    Vn[h][:, col : col + 128],
                                    pvt[:, h * D : (h + 1) * D],
                                )

                for tb in range(NTB):
                    sl = slice(tb * TB, (tb + 1) * TB)
                    chs = xch.tile([128, NCH, TB], bf16, tag="xch", bufs=3)
                    nc.sync.dma_start(
                        chs[:], kT[b, :, sl].rearrange("(c p) n -> p c n", p=128)
                    )
                    pk = ps.tile([128, 2 * TB], f32, tag="pS", name="pk", bufs=2)
                    for h in range(HPC):
                        for c in range(NCH):
                            nc.tensor.matmul(
                                pk[:, h * TB : (h + 1) * TB],
                                wk_sb[:, h, c, :], chs[:, c, :],
                                start=(c == 0), stop=(c == NCH - 1),
                            )
                    for h in range(HPC):
                        with nc.allow_low_precision(reason="f32r PE operand"):
                            nc.vector.tensor_scalar_add(
                                KTs[h][:, sl], pk[:, h * TB : (h + 1) * TB],
                                bk_sb[:, h : h + 1],
                            )


            def attn_batch(b):
                QT, KTs, Vn = QTd.pop(b), KTd.pop(b), Vnd.pop(b)
                # ---- attention: qblock pairs share 2-bank psum + one wide exp ----
                for qbp in range(NQB // 2):
                    q0 = qbp * 2 * QB
                    sl0 = slice(q0, q0 + QB)
                    sl1 = slice(q0 + QB, q0 + 2 * QB)
                    pctxs, accs, sts = [], [], []
                    for h in range(HPC):
                        pctx0 = ps.tile([128, QB], f32, tag="pC", name="pctx0", bufs=4)
                        pctx1 = ps.tile([128, QB], f32, tag="pC", name="pctx1", bufs=4)
                        pctxs.append((pctx0, pctx1))
                        accs.append(
                            work.tile([128, 2 * QB], bf16, tag=f"acc{h}", name=f"acc{h}")
                        )
                        sts.append([True, None])
                    # interleave both heads' kt iterations to hide chain latency
                    for kt in range(NKT):
                        ksl = slice(kt * 128, (kt + 1) * 128)
                        for h in range(HPC):
                            ps2 = ps.tile([128, 2 * QB], f32, tag="pS", name="ps2", bufs=2)
                            nc.tensor.matmul(
                                ps2[:, :QB], KTs[h][:, ksl], QT[h][:, sl0],
                                start=True, stop=True,
                            )
                            nc.tensor.matmul(
                                ps2[:, QB:], KTs[h][:, ksl], QT[h][:, sl1],
                                start=True, stop=True,
                            )
                            pexp = pexpp.tile([128, 2 * QB], bf16, tag="pexp", bufs=8)
                            nc.scalar.activation(pexp[:], ps2[:], AF.Exp, scale=scale)
                            nc.tensor.matmul(
                                pctxs[h][0][:], Vn[h][:, ksl], pexp[:, :QB],
                                start=(kt == 0), stop=(kt == NKT - 1),
                            )
                            nc.tensor.matmul(
                                pctxs[h][1][:], Vn[h][:, ksl], pexp[:, QB:],
                                start=(kt == 0), stop=(kt == NKT - 1),
                            )
                            st = sts[h]
                            with nc.allow_low_precision(reason="bf16 rowsum acc"):
                                if st[0] and st[1] is None:
                                    st[1] = pexp
                                elif st[0]:
                                    nc.vector.tensor_add(accs[h][:], st[1][:], pexp[:])
                                    st[0] = False
                                else:
                                    nc.vector.tensor_add(accs[h][:], accs[h][:], pexp[:])
                    # normalize both heads (frees pctx slots for pz reuse)
                    ctxns = []
                    for h in range(HPC):
                        pbc = ps.tile([128, 2 * QB], f32, tag="pS", name="pbc", bufs=2)
                        for hsl in (slice(0, QB), slice(QB, 2 * QB)):
                            nc.tensor.matmul(
                                pbc[:, hsl], onemb_sb[:], accs[h][:, hsl],
                                start=True, stop=True,
                            )
                        rsbr = work.tile([128, 2 * QB], f32, tag="rsbr", name="rsbr", bufs=1)
                        nc.vector.reciprocal_approx_fast(out=rsbr[:], in_=pbc[:])
                        ctxn = work.tile([128, 2 * QB], f32r, tag="ctxn", name="ctxn")
                        with nc.allow_low_precision(reason="f32r PE operand"):
                            nc.vector.tensor_mul(ctxn[:, :QB], pctxs[h][0][:], rsbr[:, :QB])
                            nc.vector.tensor_mul(ctxn[:, QB:], pctxs[h][1][:], rsbr[:, QB:])
                        ctxns.append(ctxn)
                    pzs = [
                        ps.tile([128, QB], f32, tag="pC", name="pz", bufs=4)
                        for _ in range(2)
                    ]
                    for h in range(HPC):
                        nc.tensor.matmul(
                            pzs[0][:], wh_sb[:, h, :], ctxns[h][:, :QB],
                            start=(h == 0), stop=False,
                        )
                        nc.tensor.matmul(
                            pzs[1][:], wh_sb[:, h, :], ctxns[h][:, QB:],
                            start=(h == 0), stop=False,
                        )
                    for half in range(2):
                        nc.tensor.matmul(
                            pzs[half][:], biasv_sb[:], oner_sb[:], start=False, stop=True
                        )
                        ytile = work.tile([128, QB], f32, tag="ytile")
                        nc.vector.tensor_copy(ytile[:], pzs[half][:])
                        nc.sync.dma_start(
                            y_bounce[b][qbp][:, half * QB : (half + 1) * QB], ytile[:]
                        )
                    nc.gpsimd.collective_compute(
                        "ReduceScatter",
                        mybir.AluOpType.add,
                        replica_groups=[list(range(NCORES))],
                        ins=[y_bounce[b][qbp][:].opt()],
                        outs=[y_shard[b][qbp][:].opt()],
                    )
                    nc.sync.dma_start(
                        out_y[:, b * S + qbp * (S // 2) : b * S + (qbp + 1) * (S // 2)],
                        y_shard[b][qbp][:],
                    )

            for b in range(B):
                proj_batch(b)
                if b > 0:
                    attn_batch(b - 1)
            attn_batch(B - 1)

    nc.compile()
    return nc


def kernel(**inputs):
    query = np.asarray(inputs["query"], np.float32)
    key = np.asarray(inputs["key"], np.float32)
    Wq, bq = np.asarray(inputs["Wq"], np.float32), np.asarray(inputs["bq"], np.float32)
    Wk, bk = np.asarray(inputs["Wk"], np.float32), np.asarray(inputs["bk"], np.float32)
    Wv, bv = np.asarray(inputs["Wv"], np.float32), np.asarray(inputs["bv"], np.float32)
    Wp, bp = np.asarray(inputs["Wp"], np.float32), np.asarray(inputs["bp"], np.float32)
    Wo, bo = np.asarray(inputs["Wo"], np.float32), np.asarray(inputs["bo"], np.float32)

    qT_b16 = np.ascontiguousarray(query.transpose(0, 2, 1)).astype(ml_dtypes.bfloat16)
    kT_b16 = np.ascontiguousarray(key.transpose(0, 2, 1)).astype(ml_dtypes.bfloat16)

    if "nc" not in _cache:
        _cache["nc"] = build()
    nc = _cache["nc"]

    in_maps = []
    for i in range(NCORES):
        hs = slice(i * HPC, (i + 1) * HPC)
        Wo_h = Wo.reshape(H, D, D)  # rows of Wo per head
        wh = np.einsum(
            "hde,hef->hdf",
            Wp[hs].astype(np.float64),
            Wo_h[hs].astype(np.float64),
        ).astype(np.float32)
        bias = (
            np.einsum("hd,hdf->f", bv[hs].astype(np.float64), wh.astype(np.float64))
            + np.einsum(
                "hd,hdf->f", bp[hs].astype(np.float64), Wo_h[hs].astype(np.float64)
            )
            + bo.astype(np.float64) / NCORES
        ).astype(np.float32)
        in_maps.append(
            {
                "qT": qT_b16,
                "kT": kT_b16,
                "wq": np.ascontiguousarray(Wq[hs]).astype(ml_dtypes.bfloat16),
                "wk": np.ascontiguousarray(Wk[hs]).astype(ml_dtypes.bfloat16),
                "wv": np.ascontiguousarray(Wv[hs]).astype(ml_dtypes.bfloat16),
                "wh": wh,
                "bqT": np.ascontiguousarray(bq[hs].T),
                "bkT": np.ascontiguousarray(bk[hs].T),
                "biasv": bias.reshape(1, D),
                "oner": np.ones((1, QB), np.float32),
                "onemb": np.ones((D, D), ml_dtypes.bfloat16),
            }
        )

    res = run_bass_kernel_spmd(nc, in_maps, core_ids=list(range(NCORES)))
    _cache["last_result"] = res
    yT = np.concatenate([res.results[i]["out_y"] for i in range(NCORES)], axis=0)
    return np.ascontiguousarray(yT.T).reshape(B, S, D)


# revision 31
# speedup vs baseline: 1.0167x; 1.0167x over previous
"""Multi-head attention on 8 trn2 NeuronCores, head-parallel (2 heads/core).

Math per head h (reference semantics):
  Q = query @ Wq[h] + bq[h];  K = key @ Wk[h] + bk[h];  V = query @ Wv[h] + bv[h]
  P = exp(Q K^T / sqrt(D));  alpha = P / rowsum(P)
  ctx = alpha @ V;  y_h = (ctx @ Wp[h] + bp[h]) @ Wo[h]
  out = sum_h y_h + bo

Device-side formulation (all layouts transposed, f32 storage, f32r matmuls):
  Per core: project QT/KT/VT = W^T @ xT per head, attention with unnormalized
  softmax (rowsum via ones-matmul, normalization folded after PV), output
  y^T[e, tok] partial = sum_{h in core} W_h^T @ ctxn_h + bias ** ones, where
  W_h = Wp[h] @ Wo[h] (host-premultiplied) and bias collects bv/bp/bo terms.
  ReduceScatter across the 8 cores; host concatenates the shards.
"""

import sys

if "/opt/trn_rl_repo" not in sys.path:
    sys.path.insert(0, "/opt/trn_rl_repo")

import ml_dtypes
import numpy as np

import concourse.mybir as mybir
import concourse.tile as tile
from concourse import bacc
from concourse.bass_utils import run_bass_kernel_spmd

B, S = 4, 2048
IN, D, H = 1024, 128, 16
NCORES = 8
HPC = H // NCORES  # heads per core
NCH = IN // 128  # input chunks
TB = 512  # projection token block
NTB = S // TB
QB = 512  # attention query block
NQB = S // QB
KT = 128  # attention key tile
NKT = S // KT
ESH = D // NCORES  # output shard rows per core

f32 = mybir.dt.float32
f32r = mybir.dt.float32r
bf16 = mybir.dt.bfloat16
AF = mybir.ActivationFunctionType

_cache = {}


def build():
    nc = bacc.Bacc(None, target_bir_lowering=False, num_devices=NCORES)

    qT = nc.dram_tensor("qT", [B, IN, S], bf16, kind="ExternalInput")
    kT = nc.dram_tensor("kT", [B, IN, S], bf16, kind="ExternalInput")
    wq = nc.dram_tensor("wq", [HPC, IN, D], bf16, kind="ExternalInput")
    wk = nc.dram_tensor("wk", [HPC, IN, D], bf16, kind="ExternalInput")
    wv = nc.dram_tensor("wv", [HPC, IN, D], bf16, kind="ExternalInput")
    wh = nc.dram_tensor("wh", [HPC, D, D], f32r, kind="ExternalInput")
    bqT = nc.dram_tensor("bqT", [D, HPC], f32, kind="ExternalInput")
    bkT = nc.dram_tensor("bkT", [D, HPC], f32, kind="ExternalInput")
    biasv = nc.dram_tensor("biasv", [1, D], f32r, kind="ExternalInput")
    oner = nc.dram_tensor("oner", [1, QB], f32r, kind="ExternalInput")
    onemb = nc.dram_tensor("onemb", [D, D], bf16, kind="ExternalInput")

    out_y = nc.dram_tensor("out_y", [ESH, B * S], f32, kind="ExternalOutput")
    y_bounce = [
        [nc.dram_tensor(f"y_bounce{b}_{q}", [D, S // 2], f32) for q in range(2)]
        for b in range(B)
    ]
    y_shard = [
        [nc.dram_tensor(f"y_shard{b}_{q}", [ESH, S // 2], f32) for q in range(2)]
        for b in range(B)
    ]

    scale = 1.0 / float(np.sqrt(D))

    with tile.TileContext(nc) as tc:
        with (
            tc.tile_pool(name="const", bufs=1) as cpool,
            tc.tile_pool(name="xch", bufs=24) as xch,
            tc.tile_pool(name="qkv", bufs=2) as qkv,
            tc.tile_pool(name="work", bufs=2) as work,
            tc.tile_pool(name="pexpp", bufs=4) as pexpp,
            tc.tile_pool(name="ps", bufs=2, space="PSUM") as ps,
        ):
            # ---- resident constants ----
            wq_sb = cpool.tile([128, HPC, NCH, D], bf16, tag="wq_sb")
            wk_sb = cpool.tile([128, HPC, NCH, D], bf16, tag="wk_sb")
            wv_sb = cpool.tile([128, HPC, NCH, D], bf16, tag="wv_sb")
            for sb_t, dram_t in ((wq_sb, wq), (wk_sb, wk), (wv_sb, wv)):
                nc.sync.dma_start(
                    sb_t[:], dram_t[:].rearrange("h (c p) d -> p h c d", p=128)
                )
            wh_sb = cpool.tile([128, HPC, D], f32r, tag="wh_sb")
            nc.sync.dma_start(wh_sb[:], wh[:].rearrange("h d e -> d h e"))
            bq_sb = cpool.tile([128, HPC], f32, tag="bq_sb")
            bk_sb = cpool.tile([128, HPC], f32, tag="bk_sb")
            nc.sync.dma_start(bq_sb[:], bqT[:])
            nc.sync.dma_start(bk_sb[:], bkT[:])
            biasv_sb = cpool.tile([1, D], f32r, tag="biasv_sb")
            oner_sb = cpool.tile([1, QB], f32r, tag="oner_sb")
            onemb_sb = cpool.tile([D, D], bf16, tag="onemb_sb")
            nc.sync.dma_start(onemb_sb[:], onemb[:])
            nc.sync.dma_start(biasv_sb[:], biasv[:])
            nc.sync.dma_start(oner_sb[:], oner[:])

            QTd, KTd, Vnd = {}, {}, {}

            def proj_batch(b):
                # ---- projections: Q & V from qT, K from kT ----
                QT = QTd[b] = [qkv.tile([128, S], bf16, tag=f"QT{h}", name=f"QT{h}") for h in range(HPC)]
                KTs = KTd[b] = [qkv.tile([128, S], bf16, tag=f"KT{h}", name=f"KT{h}") for h in range(HPC)]
                Vn = Vnd[b] = [qkv.tile([128, S], bf16, tag=f"VN{h}", name=f"VN{h}") for h in range(HPC)]

                for tb in range(NTB):
                    sl = slice(tb * TB, (tb + 1) * TB)
                    chs = xch.tile([128, NCH, TB], bf16, tag="xch", bufs=3)
                    nc.sync.dma_start(
                        chs[:], qT[b, :, sl].rearrange("(c p) n -> p c n", p=128)
                    )
                    pq = ps.tile([128, 2 * TB], f32, tag="pS", name="pq", bufs=2)
                    for h in range(HPC):
                        for c in range(NCH):
                            nc.tensor.matmul(
                                pq[:, h * TB : (h + 1) * TB],
                                wq_sb[:, h, c, :], chs[:, c, :],
                                start=(c == 0), stop=(c == NCH - 1),
                            )
                    for h in range(HPC):
                        with nc.allow_low_precision(reason="f32r PE operand"):
                            nc.vector.tensor_scalar_add(
                                QT[h][:, sl], pq[:, h * TB : (h + 1) * TB],
                                bq_sb[:, h : h + 1],
                            )
                    # V in natural [tok, d] layout: chunk subtiles as stationary
                    for t in range(TB // 128):
                        pvt = ps.tile([128, 2 * D], f32, tag="pC", name="pvt", bufs=4)
                        for c in range(NCH):
                            nc.tensor.matmul(
                                pvt[:],
                                chs[:, c, t * 128 : (t + 1) * 128],
                                wv_sb[:, :, c, :],
                                start=(c == 0), stop=(c == NCH - 1),
                            )
                        col = tb * TB + t * 128
                        for h in range(HPC):
                            with nc.allow_low_precision(reason="bf16 PV operand"):
                                nc.vector.tensor_copy(
                                    Vn[h][:, col : col + 128],
                                    pvt[:, h * D : (h + 1) * D],
                                )

                for tb in range(NTB):
                    sl = slice(tb * TB, (tb + 1) * TB)
                    chs = xch.tile([128, NCH, TB], bf16, tag="xch", bufs=3)
                    nc.sync.dma_start(
                        chs[:], kT[b, :, sl].rearrange("(c p) n -> p c n", p=128)
                    )
                    pk = ps.tile([128, 2 * TB], f32, tag="pS", name="pk", bufs=2)
                    for h in range(HPC):
                        for c in range(NCH):
                            nc.tensor.matmul(
                                pk[:, h * TB : (h + 1) * TB],
                                wk_sb[:, h, c, :], chs[:, c, :],
                                start=(c == 0), stop=(c == NCH - 1),
                            )
                    for h in range(HPC):
                        with nc.allow_low_precision(reason="f32r PE operand"):
                            nc.vector.tensor_scalar_add(
                                KTs[h][:, sl], pk[:, h * TB : (h + 1) * TB],
                                bk_sb[:, h : h + 1],
                            )


            def attn_batch(b):
                QT, KTs, Vn = QTd.pop(b), KTd.pop(b), Vnd.pop(b)
                # ---- attention: qblock pairs share 2-bank psum + one wide exp ----
                for qbp in range(NQB // 2):
                    q0 = qbp * 2 * QB
                    sl0 = slice(q0, q0 + QB)
                    sl1 = slice(q0 + QB, q0 + 2 * QB)
                    pctxs, accs = [], []
                    for h in range(HPC):
                        pctx0 = ps.tile([128, QB], f32, tag="pC", name="pctx0", bufs=4)
                        pctx1 = ps.tile([128, QB], f32, tag="pC", name="pctx1", bufs=4)
                        acc_d = work.tile([128, 2 * QB], bf16, tag="acc_d", name="acc_d")
                        st = [True, None]
                        for kt in range(NKT):
                            ps2 = ps.tile([128, 2 * QB], f32, tag="pS", name="ps2", bufs=2)
                            ksl = slice(kt * 128, (kt + 1) * 128)
                            nc.tensor.matmul(
                                ps2[:, :QB], KTs[h][:, ksl], QT[h][:, sl0],
                                start=True, stop=True,
                            )
                            nc.tensor.matmul(
                                ps2[:, QB:], KTs[h][:, ksl], QT[h][:, sl1],
                                start=True, stop=True,
                            )
                            pexp = pexpp.tile([128, 2 * QB], bf16, tag="pexp", bufs=8)
                            nc.scalar.activation(pexp[:], ps2[:], AF.Exp, scale=scale)
                            nc.tensor.matmul(
                                pctx0[:], Vn[h][:, ksl], pexp[:, :QB],
                                start=(kt == 0), stop=(kt == NKT - 1),
                            )
                            nc.tensor.matmul(
                                pctx1[:], Vn[h][:, ksl], pexp[:, QB:],
                                start=(kt == 0), stop=(kt == NKT - 1),
                            )
                            with nc.allow_low_precision(reason="bf16 rowsum acc"):
                                if st[0] and st[1] is None:
                                    st[1] = pexp
                                elif st[0]:
                                    nc.vector.tensor_add(acc_d[:], st[1][:], pexp[:])
                                    st[0] = False
                                else:
                                    nc.vector.tensor_add(acc_d[:], acc_d[:], pexp[:])
                        pctxs.append((pctx0, pctx1))
                        accs.append(acc_d)
                    # normalize both heads (frees pctx slots for pz reuse)
                    ctxns = []
                    for h in range(HPC):
                        pbc = ps.tile([128, 2 * QB], f32, tag="pS", name="pbc", bufs=2)
                        for hsl in (slice(0, QB), slice(QB, 2 * QB)):
                            nc.tensor.matmul(
                                pbc[:, hsl], onemb_sb[:], accs[h][:, hsl],
                                start=True, stop=True,
                            )
                        rsbr = work.tile([128, 2 * QB], f32, tag="rsbr", name="rsbr", bufs=1)
                        nc.vector.reciprocal_approx_fast(out=rsbr[:], in_=pbc[:])
                        ctxn = work.tile([128, 2 * QB], f32r, tag="ctxn", name="ctxn")
                        with nc.allow_low_precision(reason="f32r PE operand"):
                            nc.vector.tensor_mul(ctxn[:, :QB], pctxs[h][0][:], rsbr[:, :QB])
                            nc.vector.tensor_mul(ctxn[:, QB:], pctxs[h][1][:], rsbr[:, QB:])
                        ctxns.append(ctxn)
                    pzs = [
                        ps.tile([128, QB], f32, tag="pC", name="pz", bufs=4)
                        for _ in range(2)
                    ]
                    for h in range(HPC):
                        nc.tensor.matmul(
                            pzs[0][:], wh_sb[:, h, :], ctxns[h][:, :QB],
                            start=(h == 0), stop=False,
                        )
                        nc.tensor.matmul(
                            pzs[1][:], wh_sb[:, h, :], ctxns[h][:, QB:],
                            start=(h == 0), stop=False,
                        )
                    for half in range(2):
                        nc.tensor.matmul(
                            pzs[half][:], biasv_sb[:], oner_sb[:], start=False, stop=True
                        )
                        ytile = work.tile([128, QB], f32, tag="ytile")
                        nc.vector.tensor_copy(ytile[:], pzs[half][:])
                        nc.sync.dma_start(
                            y_bounce[b][qbp][:, half * QB : (half + 1) * QB], ytile[:]
                        )
                    nc.gpsimd.collective_compute(
                        "ReduceScatter",
                        mybir.AluOpType.add,
                        replica_groups=[list(range(NCORES))],
                        ins=[y_bounce[b][qbp][:].opt()],
                        outs=[y_shard[b][qbp][:].opt()],
                    )
                    nc.sync.dma_start(
                        out_y[:, b * S + qbp * (S // 2) : b * S + (qbp + 1) * (S // 2)],
                        y_shard[b][qbp][:],
                    )

            for b in range(B):
                proj_batch(b)
                if b > 0:
                    attn_batch(b - 1)
            attn_batch(B - 1)

    nc.compile()
    return nc


def kernel(**inputs):
    query = np.asarray(inputs["query"], np.float32)
    key = np.asarray(inputs["key"], np.float32)
    Wq, bq = np.asarray(inputs["Wq"], np.float32), np.asarray(inputs["bq"], np.float32)
    Wk, bk = np.asarray(inputs["Wk"], np.float32), np.asarray(inputs["bk"], np.float32)
    Wv, bv = np.asarray(inputs["Wv"], np.float32), np.asarray(inputs["bv"], np.float32)
    Wp, bp = np.asarray(inputs["Wp"], np.float32), np.asarray(inputs["bp"], np.float32)
    Wo, bo = np.asarray(inputs["Wo"], np.float32), np.asarray(inputs["bo"], np.float32)

    qT_b16 = np.ascontiguousarray(query.transpose(0, 2, 1)).astype(ml_dtypes.bfloat16)
    kT_b16 = np.ascontiguousarray(key.transpose(0, 2, 1)).astype(ml_dtypes.bfloat16)

    if "nc" not in _cache:
        _cache["nc"] = build()
    nc = _cache["nc"]

    in_maps = []
    for i in range(NCORES):
        hs = slice(i * HPC, (i + 1) * HPC)
        Wo_h = Wo.reshape(H, D, D)  # rows of Wo per head
        wh = np.einsum(
            "hde,hef->hdf",
            Wp[hs].astype(np.float64),
            Wo_h[hs].astype(np.float64),
        ).astype(np.float32)
        bias = (
            np.einsum("hd,hdf->f", bv[hs].astype(np.float64), wh.astype(np.float64))
            + np.einsum(
                "hd,hdf->f", bp[hs].astype(np.float64), Wo_h[hs].astype(np.float64)
            )
            + bo.astype(np.float64) / NCORES
        ).astype(np.float32)
        in_maps.append(
            {
                "qT": qT_b16,
                "kT": kT_b16,
                "wq": np.ascontiguousarray(Wq[hs]).astype(ml_dtypes.bfloat16),
                "wk": np.ascontiguousarray(Wk[hs]).astype(ml_dtypes.bfloat16),
                "wv": np.ascontiguousarray(Wv[hs]).astype(ml_dtypes.bfloat16),
                "wh": wh,
                "bqT": np.ascontiguousarray(bq[hs].T),
                "bkT": np.ascontiguousarray(bk[hs].T),
                "biasv": bias.reshape(1, D),
                "oner": np.ones((1, QB), np.float32),
                "onemb": np.ones((D, D), ml_dtypes.bfloat16),
            }
        )

    res = run_bass_kernel_spmd(nc, in_maps, core_ids=list(range(NCORES)))
    _cache["last_result"] = res
    yT = np.concatenate([res.results[i]["out_y"] for i in range(NCORES)], axis=0)
    return np.ascontiguousarray(yT.T).reshape(B, S, D)
